# revision 31
# baseline (speedup 1.0000x reference)
"""Trainium2 Bass kernel for memory-augmented causal attention.

Reference computation (b=2, n=1024, m=1024 memory, 16 heads, d_head=64):
  q = (x @ Wq) * scale ; k,v = split(x @ Wkv) ; k = [mem_k; k] ; v = [mem_v; v]
  sim = q k^T + pos_bias ; causal mask on self part ; softmax ; out = attn v
  return out @ Wo + bo

Sharding: 16 heads across 8 cores (2 heads/core), both batches on every core.
Each core computes a partial output (its heads' contribution through Wo rows);
host sums the 8 partials.

Structure (wave-packed):
  - The PE array is treated as a 4x4 grid of 32x32 subarrays; matmuls whose
    (contraction-rows x output-partition) rectangles are disjoint execute
    concurrently (tile_position row/col tiling).  Per j-chunk the attention
    work is emitted as waves of concurrent MMs:
      qk wave (per b):  h0 [rows 0-63 x cols all] || h1 [rows 64-127 x ...]
        (memory chunks: h1 split jlo/jhi with the output partition halves
        swapped -- the "rotation" -- so the bias wave can 4-pack)
      bias wave (per b, memory chunks): 4 concurrent 64-row identity MMs in
        the 4 disjoint diagonal/anti-diagonal rectangles; self chunks fall
        back to 2 serial full-identity MMs (no rotation there, so the
        on-chip self-v transpose copy stays partition-aligned).
      AV wave (per b):  h0 -> av[0:64] (col strips 0-1) || h1 -> av[64:128]
      denom wave: 4 concurrent [128,1]-stationary ones-MMs at output rows
        0/32/64/96 of one PSUM bank (softmax denominators).
  - pos_bias is fp8e4 (mask=-240): half DMA, exact-enough bias add on PE.
  - Causal i-windowing: self-attention chunks skip their masked i-prefix.
  - Normalization: reciprocal of denom rows, gpsimd partition-broadcast,
    one [128,512] DVE multiply writing outT directly (no staging DMA).
  - Output projection in f16 (outT, Wo); partial output summed on host.
  - PSUM: sim ring 2x[128,1024] (4 banks) + av 2x[128,512] (2) + denom
    2x[128,512] (2); transients share the sim ring.
"""

import numpy as np

import concourse.bass as bass
import concourse.mybir as mybir
import concourse.tile as tile
from concourse import bacc
from concourse import bass_utils
from concourse.masks import make_identity

F32 = mybir.dt.float32
F32R = mybir.dt.float32r
F16 = mybir.dt.float16
BF16 = mybir.dt.bfloat16
F8E4 = mybir.dt.float8e4

HEADS = 16
DH = 64               # head dim
B = 2                 # batch
N = 1024              # query length
M = 1024              # memory length
JT = N + M            # total key length
DIM = 1024
SCALE = DH ** -0.5
NCORE = 8
HPC = HEADS // NCORE  # heads per core = 2

NKC = DIM // 128      # contraction chunks for projections = 8
NJ = JT // 128        # j chunks = 16
NJ_MEM = M // 128     # memory j chunks = 8
NIC = N // 512        # i chunks of 512 = 2

AV_DELAY = 3          # AV trails sims by this many j-chunks


def _self_chunks(ic):
    return (ic * 512 + 511) // 128 + 1


def _unmasked_jcs(ic):
    return list(range(NJ_MEM)) + [NJ_MEM + k for k in range(min(8, _self_chunks(ic)))]


WINDOWING = True


def _wlo(ic, jc):
    """first unmasked i-column (within the 512 i-chunk) for this j-chunk"""
    if not WINDOWING or jc < NJ_MEM:
        return 0
    return max(0, (jc - NJ_MEM) * 128 - ic * 512)


_NC_CACHE = None


def _build():
    global _NC_CACHE
    if _NC_CACHE is not None:
        return _NC_CACHE

    nc = bacc.Bacc("TRN2", target_bir_lowering=False, debug=False)

    XT = nc.dram_tensor("xT", [B, DIM, N], F16, kind="ExternalInput").ap()
    WQ = nc.dram_tensor("wq", [128, DIM], F16, kind="ExternalInput").ap()
    WK = nc.dram_tensor("wk", [128, DIM], F16, kind="ExternalInput").ap()
    WV = nc.dram_tensor("wv", [128, DIM], F16, kind="ExternalInput").ap()
    WO = nc.dram_tensor("wo", [128, DIM], F16, kind="ExternalInput").ap()
    MKT = nc.dram_tensor("mkT", [B, 128, M], F16, kind="ExternalInput").ap()
    MV = nc.dram_tensor("mv", [B, HPC, 128, NJ_MEM * DH], F16,
                        kind="ExternalInput").ap()
    EB0 = nc.dram_tensor("eb0", [JT, N], F8E4, kind="ExternalInput").ap()
    EBX1 = nc.dram_tensor("ebx1", [JT, N], F16, kind="ExternalInput").ap()
    OUT = nc.dram_tensor("out", [B, N, DIM], F16, kind="ExternalOutput").ap()

    with tile.TileContext(nc) as tc:
        with tc.tile_pool(name="const", bufs=1) as cp, \
             tc.tile_pool(name="wts", bufs=1) as wp, \
             tc.tile_pool(name="xtp", bufs=16) as xtp, \
             tc.tile_pool(name="big", bufs=1) as bigp, \
             tc.tile_pool(name="stage", bufs=2) as stp, \
             tc.tile_pool(name="ebp", bufs=8) as ebp, \
             tc.tile_pool(name="expp", bufs=10) as expp, \
             tc.tile_pool(name="outst", bufs=4) as outstp, \
             tc.tile_pool(name="smalls", bufs=2) as smallp, \
             tc.tile_pool(name="psum", bufs=1, space="PSUM") as psp:

            # ---- input DMAs, priority order, all on the sync HW-DGE ----
            wq_t = wp.tile([128, DIM], F16)
            wk_t = wp.tile([128, DIM], F16)
            wv_t = wp.tile([128, DIM], F16)
            wo_t = wp.tile([128, DIM], F16)
            # first kc chunk of wq lands first: matmul #1 needs only it
            nc.sync.dma_start(wq_t[:, 0:128], WQ[:, 0:128])
            nc.sync.dma_start(wq_t[:, 128:DIM], WQ[:, 128:DIM])
            nc.sync.dma_start(wk_t[:, 0:128], WK[:, 0:128])
            xbig = {}
            xts = {}
            for b in range(B):
                for g in range(NKC // 2):
                    xbig[(b, g)] = xtp.tile([128, 2 * N], F16,
                                            name=f"xb{b}_{g}", tag="xt")
                for kc in range(NKC):
                    xts[(b, kc)] = xbig[(b, kc // 2)][
                        :, (kc % 2) * N:(kc % 2 + 1) * N]

            def load_x(b, g):
                nc.sync.dma_start(
                    xbig[(b, g)][:].rearrange("p (c i) -> p c i", c=2),
                    XT[b, g * 256:(g + 1) * 256, :].rearrange(
                        "(c p) i -> p c i", c=2))

            for g in range(NKC // 2):
                load_x(0, g)
            nc.sync.dma_start(wk_t[:, 128:DIM], WK[:, 128:DIM])
            for g in range(NKC // 2):
                load_x(1, g)
            nc.sync.dma_start(wv_t[:], WV)

            qT = [bigp.tile([128, N], F16, name=f"qT{b}") for b in range(B)]
            kT = [bigp.tile([128, JT], F16, name=f"kT{b}") for b in range(B)]
            vaug = [bigp.tile([128, HPC * NJ * DH], F16, name=f"vaug{b}")
                    for b in range(B)]
            outT = [bigp.tile([128, N], F16, name=f"outT{b}") for b in range(B)]

            def vaug64(b, h, jc):
                o = (h * NJ + jc) * DH
                return vaug[b][:, o:o + DH]

            for b in range(B):
                nc.sync.dma_start(kT[b][:, 0:M], MKT[b])
            ebt = {}
            ebbig = {}
            for g in range(4):
                ebbig[(0, g)] = ebp.tile([128, 4 * N], F8E4,
                                         name=f"ebb0_{g}", tag="eb0",
                                         bufs=4)
                ebbig[(1, g)] = ebp.tile([128, 4 * N], F16,
                                         name=f"ebb1_{g}", tag="eb1",
                                         bufs=4)
            for h in range(HPC):
                for jc in range(NJ):
                    ebt[(h, jc)] = ebbig[(h, jc // 4)][
                        :, (jc % 4) * N:(jc % 4 + 1) * N]

            def load_eb(g):
                nc.sync.dma_start(
                    ebbig[(0, g)][:].rearrange("p (c i) -> p c i", c=4),
                    EB0[g * 512:(g + 1) * 512, :].rearrange(
                        "(c p) i -> p c i", c=4))
                nc.sync.dma_start(
                    ebbig[(1, g)][:].rearrange("p (c i) -> p c i", c=4),
                    EBX1[g * 512:(g + 1) * 512, :].rearrange(
                        "(c p) i -> p c i", c=4))

            load_eb(0)
            load_eb(1)
            for b in range(B):
                for h in range(HPC):
                    nc.sync.dma_start(
                        vaug[b][:, h * NJ * DH:(h * NJ + NJ_MEM) * DH],
                        MV[b, h])
            load_eb(2)
            load_eb(3)
            nc.sync.dma_start(wo_t[:], WO)

            # ---- constants (before memsets: warms gate on identh) ----
            identh = cp.tile([128, 128], F16)
            make_identity(nc, identh[:])
            identb = cp.tile([128, 128], F8E4)
            make_identity(nc, identb[:])
            # ones column for the denominator matmuls
            onesc = cp.tile([128, 1], F16)
            nc.vector.memset(onesc[:], 1.0)
            # all-ones [128, 64] for the reciprocal broadcast outer-products
            onesb = cp.tile([128, 64], F32)
            nc.vector.memset(onesb[:], 1.0)

            def warm(n):
                wps = psp.tile([128, 512], F32, name="warmps", tag="simps",
                               bufs=2)
                for _ in range(n):
                    nc.tensor.matmul(wps[:, 0:128], identh[:], identh[:],
                                     start=True, stop=True,
                                     skip_group_check=True)

            # =============== Phase A: q/k projections ===============
            def proj_qk(kind, b):
                wt = wq_t if kind == "q" else wk_t
                ps = psp.tile([128, N], F32, name="projps", tag="simps", bufs=2)
                # kc-outer: consume each xT tile as it lands (both i-halves
                # per tile) so the PE trails the DMA stream instead of
                # catching up and stalling
                for kc in range(NKC):
                    for icx in range(NIC):
                        nc.tensor.matmul(
                            ps[:, icx * 512:(icx + 1) * 512],
                            wt[:, kc * 128:(kc + 1) * 128],
                            xts[(b, kc)][:, icx * 512:(icx + 1) * 512],
                            start=(kc == 0), stop=(kc == NKC - 1))
                if kind == "q":
                    nc.vector.tensor_copy(qT[b][:], ps[:])
                else:
                    nc.vector.tensor_copy(kT[b][:, M:JT], ps[:])

            warm(24)
            for kind, b in (("q", 0), ("k", 0), ("q", 1), ("k", 1)):
                proj_qk(kind, b)

            # ---- v projection pieces (hooked under early ic0 chunks) ----
            vst = [stp.tile([128, N], F16, name=f"vstage{b}") for b in range(B)]

            def v_proj_piece(b, icx):
                ps = psp.tile([128, 512], F32, name="vps", tag="simps",
                              bufs=2)
                for kc in range(NKC):
                    nc.tensor.matmul(
                        ps[:],
                        wv_t[:, kc * 128:(kc + 1) * 128],
                        xts[(b, kc)][:, icx * 512:(icx + 1) * 512],
                        start=(kc == 0), stop=(kc == NKC - 1))
                copy_balanced(vst[b][:, icx * 512:(icx + 1) * 512], ps[:],
                              eng="v")

            def v_transpose(b):
                for jb in range(8):
                    tp = psp.tile([128, 128], F16, name="tps",
                                  tag="simps", bufs=2)
                    nc.tensor.transpose(
                        tp[:], vst[b][:, jb * 128:(jb + 1) * 128], identh[:])
                    jc = NJ_MEM + jb
                    dst = vaug[b][:].rearrange(
                        "p (hh jjc x) -> p hh jjc x", hh=HPC, x=DH)[
                        :, :, jc, :]
                    nc.vector.tensor_copy(
                        dst, tp[:].rearrange("p (hh x) -> p hh x", hh=HPC))

            copy_idx = 0
            in_tail = [False]

            def copy_balanced(out_ap, in_ap, eng=None):
                nonlocal copy_idx
                if eng is None:
                    eng = "v" if copy_idx % 2 == 0 else "s"
                    copy_idx += 1
                if eng == "v":
                    nc.vector.tensor_copy(out_ap, in_ap)
                else:
                    nc.scalar.copy(out_ap, in_ap)

            # =============== attention machinery ===============
            def normalize(av, den, ic, b):
                # denominator rows: (h, b) -> b*64 + h*32
                for h in range(HPC):
                    r = b * 64 + h * 32
                    s = smallp.tile([1, 512], F32, name=f"sums{h}")
                    nc.vector.tensor_copy(s[:], den[r:r + 1, :])
                    rc = smallp.tile([1, 512], F32, name=f"recip{h}")
                    nc.vector.reciprocal_approx_fast(rc[:], s[:])
                    rb = smallp.tile([64, 512], F32, name=f"recipb{h}")
                    nc.gpsimd.partition_broadcast(rb[:], rc[:])
                    nc.vector.tensor_tensor(
                        outT[b][h * 64:(h + 1) * 64,
                                ic * 512:(ic + 1) * 512],
                        av[b][h * 64:(h + 1) * 64, :], rb[:],
                        mybir.AluOpType.mult)

            def out_proj_half(b, ib):
                ob = outstp.tile([128, DIM], F16, name="ob")
                ps = psp.tile([128, N], F32, name="ops", tag="simps",
                              bufs=2)
                for dc in range(DIM // 512):
                    nc.tensor.matmul(
                        ps[:, dc * 512:(dc + 1) * 512],
                        outT[b][:, ib * 128:(ib + 1) * 128],
                        wo_t[:, dc * 512:(dc + 1) * 512],
                        start=True, stop=True)
                copy_balanced(ob[:], ps[:])
                nc.sync.dma_start(OUT[b, ib * 128:(ib + 1) * 128, :], ob[:])

            def emit_sims(ic, jc, jj):
                wlo = _wlo(ic, jc)
                sb = [psp.tile([128, N], F32, name=f"simps{b}", tag="simps",
                               bufs=2) for b in range(B)]
                jcl = jc * 128
                for b in range(B):
                    qsl = slice(ic * 512 + wlo, (ic + 1) * 512)
                    # wave: disjoint row strips, same tile -> co-ready pair
                    nc.tensor.matmul(sb[b][:, wlo:512],
                                     kT[b][0:64, jcl:jcl + 128],
                                     qT[b][0:64, qsl],
                                     start=True, stop=False,
                                     skip_group_check=True)
                    nc.tensor.matmul(sb[b][:, 512 + wlo:1024],
                                     kT[b][64:128, jcl:jcl + 128],
                                     qT[b][64:128, qsl],
                                     start=True, stop=True,
                                     skip_group_check=True)
                    ec = slice(ic * 512 + wlo, (ic + 1) * 512)
                    nc.tensor.matmul(sb[b][:, wlo:512], identb[:],
                                     ebt[(0, jc)][:, ec],
                                     start=False, stop=True,
                                     skip_group_check=True)
                out = []
                ec = slice(ic * 512 + wlo, (ic + 1) * 512)
                for b in range(B):
                    er = expp.tile([128, N], F16, name="expraw", tag="expraw")
                    if wlo == 0:
                        nc.scalar.activation(
                            er[:], sb[b][:], mybir.ActivationFunctionType.Exp)
                    else:
                        nc.scalar.activation(
                            er[:].rearrange("p (h i) -> p h i",
                                            h=HPC)[:, :, wlo:512],
                            sb[b][:].rearrange("p (h i) -> p h i",
                                               h=HPC)[:, :, wlo:512],
                            mybir.ActivationFunctionType.Exp)
                    # h1 bias applied multiplicatively (exp(sim)*exp(bias))
                    nc.vector.tensor_tensor(
                        er[:, 512 + wlo:1024], er[:, 512 + wlo:1024],
                        ebt[(1, jc)][:, ec], mybir.AluOpType.mult)
                    out.append(er)
                return out

            def make_av():
                avs = [psp.tile([128, 512], F32, name=f"avc{b}", tag="avps",
                                bufs=2) for b in range(B)]
                for a in avs:
                    nc.vector.memset(a[:], 0.0)
                return avs

            def make_denom():
                d = psp.tile([128, 512], F32, name="denom", tag="denom",
                             bufs=2)
                nc.vector.memset(d[:], 0.0)
                return d

            def emit_av(av, den, jc, ic, e2s, first, last):
                wlo = _wlo(ic, jc)
                for b in range(B):
                    # wave: disjoint col strips -> concurrent
                    nc.tensor.matmul(av[b][0:64, wlo:512], vaug64(b, 0, jc),
                                     e2s[b][:, wlo:512],
                                     start=False, stop=last,
                                     skip_group_check=True)
                    nc.tensor.matmul(av[b][64:128, wlo:512], vaug64(b, 1, jc),
                                     e2s[b][:, 512 + wlo:1024],
                                     start=False, stop=last,
                                     skip_group_check=True)
                # denominator wave: 4 concurrent single-row MMs
                for b in range(B):
                    for h in range(HPC):
                        r = b * 64 + h * 32
                        nc.tensor.matmul(
                            den[r:r + 1, wlo:512], onesc[:, 0:1],
                            e2s[b][:, h * 512 + wlo:(h + 1) * 512],
                            start=False, stop=last,
                            skip_group_check=True, tile_position=(0, r))

            pend = []
            done_av = 0

            def flush_av(av, den, ic, jcs, upto):
                nonlocal done_av
                while done_av < upto:
                    jc, e2s = pend[done_av]
                    emit_av(av, den, jc, ic, e2s, done_av == 0,
                            done_av == len(jcs) - 1)
                    done_av += 1

            # ---- ic = 0 (first chunks fill the exp pipeline while
            #      the v projection runs on the PE) ----
            jcs0 = _unmasked_jcs(0)
            pend = []
            done_av = 0
            av0 = make_av()
            den0 = make_denom()
            for jj, jc in enumerate(jcs0):
                pend.append((jc, emit_sims(0, jc, jj)))
                if jj == 1:
                    for b in range(B):
                        for icx in range(NIC):
                            v_proj_piece(b, icx)
                        v_transpose(b)
                if jj >= AV_DELAY:
                    flush_av(av0, den0, 0, jcs0, jj - AV_DELAY + 1)
            flush_av(av0, den0, 0, jcs0, len(jcs0))

            # ---- ic = 1 (ic0 normalize + out-proj interleaved) ----
            jcs1 = _unmasked_jcs(1)
            pend = []
            done_av = 0
            av1 = None
            den1 = None
            normalize(av0, den0, 0, 0)
            for jj, jc in enumerate(jcs1):
                pend.append((jc, emit_sims(1, jc, jj)))
                if jj == 0:
                    av1 = make_av()
                    den1 = make_denom()
                    normalize(av0, den0, 0, 1)
                if jj >= 3 and jj % 2 == 1:
                    i = (jj - 3) // 2
                    out_proj_half(i // 4, i % 4)
                if jj >= AV_DELAY:
                    flush_av(av1, den1, 1, jcs1, jj - AV_DELAY + 1)
            flush_av(av1, den1, 1, jcs1, len(jcs1))
            out_proj_half(1, 3)

            # ---- tail ----
            in_tail[0] = True
            for b in range(B):
                normalize(av1, den1, 1, b)
            for b in range(B):
                for ib in range(4, 8):
                    out_proj_half(b, ib)

    nc.compile()
    _NC_CACHE = nc
    return nc


def _prep_inputs(x, mem_k, mem_v, pos_bias, Wq, Wkv, Wo):
    """Build per-core input maps (host-side sharding)."""
    x = np.ascontiguousarray(x, dtype=np.float32)
    xT = np.ascontiguousarray(x.transpose(0, 2, 1)).astype(np.float16)

    import ml_dtypes
    pb = np.ascontiguousarray(
        pos_bias[0].transpose(0, 2, 1)).astype(np.float32)     # [16, JT, N]
    jj = np.arange(JT)[:, None]
    ii = np.arange(N)[None, :]
    mask = jj > (ii + M)
    eb0 = np.where(mask[None], np.float32(-240.0), pb).astype(
        ml_dtypes.float8_e4m3fn)
    ebx1 = np.where(mask[None], np.float32(0.0), np.exp(pb)).astype(
        np.float16)

    def shuffle_w(w):  # [1024, 128] -> [128, 1024] with kc-major columns
        return np.ascontiguousarray(
            w.reshape(NKC, 128, 128).transpose(1, 0, 2).reshape(128, DIM))

    in_maps = []
    for c in range(NCORE):
        cs = 128 * c
        wq = shuffle_w(np.asarray(Wq[:, cs:cs + 128] * SCALE)).astype(np.float16)
        wk = shuffle_w(np.asarray(Wkv[:, cs:cs + 128])).astype(np.float16)
        wv = shuffle_w(
            np.asarray(Wkv[:, DIM + cs:DIM + cs + 128])).astype(np.float16)
        wo = np.ascontiguousarray(Wo[cs:cs + 128, :]).astype(np.float16)
        mkT = np.ascontiguousarray(
            mem_k[:, :, cs:cs + 128].transpose(0, 2, 1)).astype(np.float16)
        mv_s = mem_v[:, :, cs:cs + 128].astype(np.float16).reshape(
            B, NJ_MEM, 128, HPC, DH)
        mv = np.empty((B, HPC, 128, NJ_MEM, DH), dtype=np.float16)
        for h in range(HPC):
            mv[:, h] = mv_s[:, :, :, h, :].transpose(0, 2, 1, 3)
        mv = mv.reshape(B, HPC, 128, NJ_MEM * DH)
        in_maps.append({
            "xT": xT,
            "wq": wq, "wk": wk, "wv": wv, "wo": wo,
            "mkT": mkT,
            "mv": np.ascontiguousarray(mv),
            "eb0": np.ascontiguousarray(eb0[2 * c]),
            "ebx1": np.ascontiguousarray(ebx1[2 * c + 1]),
        })
    return in_maps


def kernel(x, mem_k, mem_v, pos_bias, Wq, Wkv, Wo, bo, **_kw):
    nc = _build()
    in_maps = _prep_inputs(
        np.asarray(x), np.asarray(mem_k), np.asarray(mem_v),
        np.asarray(pos_bias), np.asarray(Wq), np.asarray(Wkv), np.asarray(Wo))
    res = bass_utils.run_bass_kernel_spmd(nc, in_maps, core_ids=list(range(NCORE)))
    out = np.zeros((B, N, DIM), dtype=np.float64)
    for r in res.results:
        out += r["out"].astype(np.float64)
    out += np.asarray(bo, dtype=np.float64)[None, None, :]
    return out.astype(np.float32)


# revision 32
# speedup vs baseline: 1.0388x; 1.0388x over previous
"""Trainium2 Bass kernel for memory-augmented causal attention.

Reference computation (b=2, n=1024, m=1024 memory, 16 heads, d_head=64):
  q = (x @ Wq) * scale ; k,v = split(x @ Wkv) ; k = [mem_k; k] ; v = [mem_v; v]
  sim = q k^T + pos_bias ; causal mask on self part ; softmax ; out = attn v
  return out @ Wo + bo

Sharding: 16 heads across 8 cores (2 heads/core), both batches on every core.
Each core computes a partial output (its heads' contribution through Wo rows);
host sums the 8 partials.

Structure (wave-packed):
  - The PE array is treated as a 4x4 grid of 32x32 subarrays; matmuls whose
    (contraction-rows x output-partition) rectangles are disjoint execute
    concurrently (tile_position row/col tiling).  Per j-chunk the attention
    work is emitted as waves of concurrent MMs:
      qk wave (per b):  h0 [rows 0-63 x cols all] || h1 [rows 64-127 x ...]
        (memory chunks: h1 split jlo/jhi with the output partition halves
        swapped -- the "rotation" -- so the bias wave can 4-pack)
      bias wave (per b, memory chunks): 4 concurrent 64-row identity MMs in
        the 4 disjoint diagonal/anti-diagonal rectangles; self chunks fall
        back to 2 serial full-identity MMs (no rotation there, so the
        on-chip self-v transpose copy stays partition-aligned).
      AV wave (per b):  h0 -> av[0:64] (col strips 0-1) || h1 -> av[64:128]
      denom wave: 4 concurrent [128,1]-stationary ones-MMs at output rows
        0/32/64/96 of one PSUM bank (softmax denominators).
  - pos_bias is fp8e4 (mask=-240): half DMA, exact-enough bias add on PE.
  - Causal i-windowing: self-attention chunks skip their masked i-prefix.
  - Normalization: reciprocal of denom rows, gpsimd partition-broadcast,
    one [128,512] DVE multiply writing outT directly (no staging DMA).
  - Output projection in f16 (outT, Wo); partial output summed on host.
  - PSUM: sim ring 2x[128,1024] (4 banks) + av 2x[128,512] (2) + denom
    2x[128,512] (2); transients share the sim ring.
"""

import numpy as np

import concourse.bass as bass
import concourse.mybir as mybir
import concourse.tile as tile
from concourse import bacc
from concourse import bass_utils
from concourse.masks import make_identity

F32 = mybir.dt.float32
F32R = mybir.dt.float32r
F16 = mybir.dt.float16
BF16 = mybir.dt.bfloat16
F8E4 = mybir.dt.float8e4

HEADS = 16
DH = 64               # head dim
B = 2                 # batch
N = 1024              # query length
M = 1024              # memory length
JT = N + M            # total key length
DIM = 1024
SCALE = DH ** -0.5
NCORE = 8
HPC = HEADS // NCORE  # heads per core = 2

NKC = DIM // 128      # contraction chunks for projections = 8
NJ = JT // 128        # j chunks = 16
NJ_MEM = M // 128     # memory j chunks = 8
NIC = N // 512        # i chunks of 512 = 2

AV_DELAY = 3          # AV trails sims by this many j-chunks


def _self_chunks(ic):
    return (ic * 512 + 511) // 128 + 1


def _unmasked_jcs(ic):
    return list(range(NJ_MEM)) + [NJ_MEM + k for k in range(min(8, _self_chunks(ic)))]


WINDOWING = True


def _wlo(ic, jc):
    """first unmasked i-column (within the 512 i-chunk) for this j-chunk"""
    if not WINDOWING or jc < NJ_MEM:
        return 0
    return max(0, (jc - NJ_MEM) * 128 - ic * 512)


_NC_CACHE = None


def _build():
    global _NC_CACHE
    if _NC_CACHE is not None:
        return _NC_CACHE

    nc = bacc.Bacc("TRN2", target_bir_lowering=False, debug=False)

    XT = nc.dram_tensor("xT", [B, DIM, N], F16, kind="ExternalInput").ap()
    WQ = nc.dram_tensor("wq", [128, DIM], F16, kind="ExternalInput").ap()
    WK = nc.dram_tensor("wk", [128, DIM], F16, kind="ExternalInput").ap()
    WV = nc.dram_tensor("wv", [128, DIM], F16, kind="ExternalInput").ap()
    WO = nc.dram_tensor("wo", [128, DIM], F16, kind="ExternalInput").ap()
    MKT = nc.dram_tensor("mkT", [B, 128, M], F16, kind="ExternalInput").ap()
    MV = nc.dram_tensor("mv", [B, HPC, 128, NJ_MEM * DH], F16,
                        kind="ExternalInput").ap()
    EB0 = nc.dram_tensor("eb0", [JT, N], F8E4, kind="ExternalInput").ap()
    EBX1 = nc.dram_tensor("ebx1", [JT, N], BF16, kind="ExternalInput").ap()
    OUT = nc.dram_tensor("out", [B, N, DIM], F16, kind="ExternalOutput").ap()

    with tile.TileContext(nc) as tc:
        with tc.tile_pool(name="const", bufs=1) as cp, \
             tc.tile_pool(name="wts", bufs=1) as wp, \
             tc.tile_pool(name="xtp", bufs=16) as xtp, \
             tc.tile_pool(name="big", bufs=1) as bigp, \
             tc.tile_pool(name="stage", bufs=2) as stp, \
             tc.tile_pool(name="ebp", bufs=8) as ebp, \
             tc.tile_pool(name="expp", bufs=10) as expp, \
             tc.tile_pool(name="outst", bufs=4) as outstp, \
             tc.tile_pool(name="smalls", bufs=2) as smallp, \
             tc.tile_pool(name="psum", bufs=1, space="PSUM") as psp:

            # ---- input DMAs, priority order, all on the sync HW-DGE ----
            wq_t = wp.tile([128, DIM], F16)
            wk_t = wp.tile([128, DIM], F16)
            wv_t = wp.tile([128, DIM], F16)
            wo_t = wp.tile([128, DIM], F16)
            # first kc chunk of wq lands first: matmul #1 needs only it
            nc.sync.dma_start(wq_t[:, 0:128], WQ[:, 0:128])
            nc.sync.dma_start(wq_t[:, 128:DIM], WQ[:, 128:DIM])
            nc.sync.dma_start(wk_t[:, 0:128], WK[:, 0:128])
            xbig = {}
            xts = {}
            for b in range(B):
                for g in range(NKC // 2):
                    xbig[(b, g)] = xtp.tile([128, 2 * N], F16,
                                            name=f"xb{b}_{g}", tag="xt")
                for kc in range(NKC):
                    xts[(b, kc)] = xbig[(b, kc // 2)][
                        :, (kc % 2) * N:(kc % 2 + 1) * N]

            def load_x(b, g):
                nc.sync.dma_start(
                    xbig[(b, g)][:].rearrange("p (c i) -> p c i", c=2),
                    XT[b, g * 256:(g + 1) * 256, :].rearrange(
                        "(c p) i -> p c i", c=2))

            for g in range(NKC // 2):
                load_x(0, g)
            nc.sync.dma_start(wk_t[:, 128:DIM], WK[:, 128:DIM])
            for g in range(NKC // 2):
                load_x(1, g)
            nc.sync.dma_start(wv_t[:], WV)

            qT = [bigp.tile([128, N], F16, name=f"qT{b}") for b in range(B)]
            kT = [bigp.tile([128, JT], F16, name=f"kT{b}") for b in range(B)]
            vaug = [bigp.tile([128, HPC * NJ * DH], F16, name=f"vaug{b}")
                    for b in range(B)]
            outT = [bigp.tile([128, N], F16, name=f"outT{b}") for b in range(B)]

            def vaug64(b, h, jc):
                o = (h * NJ + jc) * DH
                return vaug[b][:, o:o + DH]

            for b in range(B):
                nc.sync.dma_start(kT[b][:, 0:M], MKT[b])
            ebt = {}
            ebbig = {}
            for g in range(4):
                ebbig[(0, g)] = ebp.tile([128, 4 * N], F8E4,
                                         name=f"ebb0_{g}", tag="eb0",
                                         bufs=4)
                ebbig[(1, g)] = ebp.tile([128, 4 * N], BF16,
                                         name=f"ebb1_{g}", tag="eb1",
                                         bufs=4)
            for h in range(HPC):
                for jc in range(NJ):
                    ebt[(h, jc)] = ebbig[(h, jc // 4)][
                        :, (jc % 4) * N:(jc % 4 + 1) * N]

            def load_eb(g):
                nc.sync.dma_start(
                    ebbig[(0, g)][:].rearrange("p (c i) -> p c i", c=4),
                    EB0[g * 512:(g + 1) * 512, :].rearrange(
                        "(c p) i -> p c i", c=4))
                nc.sync.dma_start(
                    ebbig[(1, g)][:].rearrange("p (c i) -> p c i", c=4),
                    EBX1[g * 512:(g + 1) * 512, :].rearrange(
                        "(c p) i -> p c i", c=4))

            load_eb(0)
            load_eb(1)
            for b in range(B):
                for h in range(HPC):
                    nc.sync.dma_start(
                        vaug[b][:, h * NJ * DH:(h * NJ + NJ_MEM) * DH],
                        MV[b, h])
            load_eb(2)
            load_eb(3)
            nc.sync.dma_start(wo_t[:], WO)

            # ---- constants (before memsets: warms gate on identh) ----
            identh = cp.tile([128, 128], F16)
            make_identity(nc, identh[:])
            identb = cp.tile([128, 128], F8E4)
            make_identity(nc, identb[:])
            # ones column for the denominator matmuls
            onesc = cp.tile([128, 1], F16)
            nc.vector.memset(onesc[:], 1.0)
            # all-ones [128, 64] for the reciprocal broadcast outer-products
            onesb = cp.tile([128, 64], F32)
            nc.vector.memset(onesb[:], 1.0)

            def warm(n):
                wps = psp.tile([128, 512], F32, name="warmps", tag="simps",
                               bufs=2)
                for _ in range(n):
                    nc.tensor.matmul(wps[:, 0:128], identh[:], identh[:],
                                     start=True, stop=True,
                                     skip_group_check=True)

            # =============== Phase A: q/k projections ===============
            def proj_qk(kind, b):
                wt = wq_t if kind == "q" else wk_t
                ps = psp.tile([128, N], F32, name="projps", tag="simps", bufs=2)
                # kc-outer: consume each xT tile as it lands (both i-halves
                # per tile) so the PE trails the DMA stream instead of
                # catching up and stalling
                for kc in range(NKC):
                    for icx in range(NIC):
                        nc.tensor.matmul(
                            ps[:, icx * 512:(icx + 1) * 512],
                            wt[:, kc * 128:(kc + 1) * 128],
                            xts[(b, kc)][:, icx * 512:(icx + 1) * 512],
                            start=(kc == 0), stop=(kc == NKC - 1))
                if kind == "q":
                    nc.vector.tensor_copy(qT[b][:], ps[:])
                else:
                    nc.vector.tensor_copy(kT[b][:, M:JT], ps[:])

            warm(24)
            for kind, b in (("q", 0), ("k", 0), ("q", 1), ("k", 1)):
                proj_qk(kind, b)

            # ---- v projection pieces (hooked under early ic0 chunks) ----
            vst = [stp.tile([128, N], F16, name=f"vstage{b}") for b in range(B)]

            def v_proj_piece(b, icx):
                ps = psp.tile([128, 512], F32, name="vps", tag="simps",
                              bufs=2)
                for kc in range(NKC):
                    nc.tensor.matmul(
                        ps[:],
                        wv_t[:, kc * 128:(kc + 1) * 128],
                        xts[(b, kc)][:, icx * 512:(icx + 1) * 512],
                        start=(kc == 0), stop=(kc == NKC - 1))
                copy_balanced(vst[b][:, icx * 512:(icx + 1) * 512], ps[:],
                              eng="v")

            def v_transpose(b):
                for jb in range(8):
                    tp = psp.tile([128, 128], F16, name="tps",
                                  tag="simps", bufs=2)
                    nc.tensor.transpose(
                        tp[:], vst[b][:, jb * 128:(jb + 1) * 128], identh[:])
                    jc = NJ_MEM + jb
                    dst = vaug[b][:].rearrange(
                        "p (hh jjc x) -> p hh jjc x", hh=HPC, x=DH)[
                        :, :, jc, :]
                    nc.vector.tensor_copy(
                        dst, tp[:].rearrange("p (hh x) -> p hh x", hh=HPC))

            copy_idx = 0
            in_tail = [False]

            def copy_balanced(out_ap, in_ap, eng=None):
                nonlocal copy_idx
                if eng is None:
                    eng = "v" if copy_idx % 2 == 0 else "s"
                    copy_idx += 1
                if eng == "v":
                    nc.vector.tensor_copy(out_ap, in_ap)
                else:
                    nc.scalar.copy(out_ap, in_ap)

            # =============== attention machinery ===============
            def normalize(av, den, ic, b):
                # denominator rows: (h, b) -> b*64 + h*32
                for h in range(HPC):
                    r = b * 64 + h * 32
                    s = smallp.tile([1, 512], F32, name=f"sums{h}")
                    nc.vector.tensor_copy(s[:], den[r:r + 1, :])
                    rc = smallp.tile([1, 512], F32, name=f"recip{h}")
                    nc.vector.reciprocal_approx_fast(rc[:], s[:])
                    rb = smallp.tile([64, 512], F32, name=f"recipb{h}")
                    nc.gpsimd.partition_broadcast(rb[:], rc[:])
                    nc.vector.tensor_tensor(
                        outT[b][h * 64:(h + 1) * 64,
                                ic * 512:(ic + 1) * 512],
                        av[b][h * 64:(h + 1) * 64, :], rb[:],
                        mybir.AluOpType.mult)

            def out_proj_half(b, ib):
                ob = outstp.tile([128, DIM], F16, name="ob")
                ps = psp.tile([128, N], F32, name="ops", tag="simps",
                              bufs=2)
                for dc in range(DIM // 512):
                    nc.tensor.matmul(
                        ps[:, dc * 512:(dc + 1) * 512],
                        outT[b][:, ib * 128:(ib + 1) * 128],
                        wo_t[:, dc * 512:(dc + 1) * 512],
                        start=True, stop=True)
                copy_balanced(ob[:], ps[:])
                nc.sync.dma_start(OUT[b, ib * 128:(ib + 1) * 128, :], ob[:])

            def emit_sims(ic, jc, jj):
                wlo = _wlo(ic, jc)
                sb = [psp.tile([128, N], F32, name=f"simps{b}", tag="simps",
                               bufs=2) for b in range(B)]
                jcl = jc * 128
                for b in range(B):
                    qsl = slice(ic * 512 + wlo, (ic + 1) * 512)
                    # wave: disjoint row strips, same tile -> co-ready pair
                    nc.tensor.matmul(sb[b][:, wlo:512],
                                     kT[b][0:64, jcl:jcl + 128],
                                     qT[b][0:64, qsl],
                                     start=True, stop=False,
                                     skip_group_check=True)
                    nc.tensor.matmul(sb[b][:, 512 + wlo:1024],
                                     kT[b][64:128, jcl:jcl + 128],
                                     qT[b][64:128, qsl],
                                     start=True, stop=True,
                                     skip_group_check=True)
                    ec = slice(ic * 512 + wlo, (ic + 1) * 512)
                    nc.tensor.matmul(sb[b][:, wlo:512], identb[:],
                                     ebt[(0, jc)][:, ec],
                                     start=False, stop=True,
                                     skip_group_check=True)
                out = []
                ec = slice(ic * 512 + wlo, (ic + 1) * 512)
                for b in range(B):
                    er = expp.tile([128, N], F16, name="expraw", tag="expraw")
                    if wlo == 0:
                        nc.scalar.activation(
                            er[:], sb[b][:], mybir.ActivationFunctionType.Exp)
                    else:
                        nc.scalar.activation(
                            er[:].rearrange("p (h i) -> p h i",
                                            h=HPC)[:, :, wlo:512],
                            sb[b][:].rearrange("p (h i) -> p h i",
                                               h=HPC)[:, :, wlo:512],
                            mybir.ActivationFunctionType.Exp)
                    # h1 bias applied multiplicatively (exp(sim)*exp(bias))
                    nc.vector.tensor_tensor(
                        er[:, 512 + wlo:1024], er[:, 512 + wlo:1024],
                        ebt[(1, jc)][:, ec], mybir.AluOpType.mult)
                    out.append(er)
                return out

            def make_av():
                avs = [psp.tile([128, 512], F32, name=f"avc{b}", tag="avps",
                                bufs=2) for b in range(B)]
                for a in avs:
                    nc.vector.memset(a[:], 0.0)
                return avs

            def make_denom():
                d = psp.tile([128, 512], F32, name="denom", tag="denom",
                             bufs=2)
                nc.vector.memset(d[:], 0.0)
                return d

            def emit_av(av, den, jc, ic, e2s, first, last):
                wlo = _wlo(ic, jc)
                for b in range(B):
                    # wave: disjoint col strips -> concurrent
                    nc.tensor.matmul(av[b][0:64, wlo:512], vaug64(b, 0, jc),
                                     e2s[b][:, wlo:512],
                                     start=False, stop=last,
                                     skip_group_check=True)
                    nc.tensor.matmul(av[b][64:128, wlo:512], vaug64(b, 1, jc),
                                     e2s[b][:, 512 + wlo:1024],
                                     start=False, stop=last,
                                     skip_group_check=True)
                # denominator wave: 4 concurrent single-row MMs
                for b in range(B):
                    for h in range(HPC):
                        r = b * 64 + h * 32
                        nc.tensor.matmul(
                            den[r:r + 1, wlo:512], onesc[:, 0:1],
                            e2s[b][:, h * 512 + wlo:(h + 1) * 512],
                            start=False, stop=last,
                            skip_group_check=True, tile_position=(0, r))

            pend = []
            done_av = 0

            def flush_av(av, den, ic, jcs, upto):
                nonlocal done_av
                while done_av < upto:
                    jc, e2s = pend[done_av]
                    emit_av(av, den, jc, ic, e2s, done_av == 0,
                            done_av == len(jcs) - 1)
                    done_av += 1

            # ---- ic = 0 (first chunks fill the exp pipeline while
            #      the v projection runs on the PE) ----
            jcs0 = _unmasked_jcs(0)
            pend = []
            done_av = 0
            av0 = make_av()
            den0 = make_denom()
            for jj, jc in enumerate(jcs0):
                pend.append((jc, emit_sims(0, jc, jj)))
                if jj == 1:
                    for b in range(B):
                        for icx in range(NIC):
                            v_proj_piece(b, icx)
                        v_transpose(b)
                if jj >= AV_DELAY:
                    flush_av(av0, den0, 0, jcs0, jj - AV_DELAY + 1)
            flush_av(av0, den0, 0, jcs0, len(jcs0))

            # ---- ic = 1 (ic0 normalize + out-proj interleaved) ----
            jcs1 = _unmasked_jcs(1)
            pend = []
            done_av = 0
            av1 = None
            den1 = None
            normalize(av0, den0, 0, 0)
            for jj, jc in enumerate(jcs1):
                pend.append((jc, emit_sims(1, jc, jj)))
                if jj == 0:
                    av1 = make_av()
                    den1 = make_denom()
                    normalize(av0, den0, 0, 1)
                if jj >= 3 and jj % 2 == 1:
                    i = (jj - 3) // 2
                    out_proj_half(i // 4, i % 4)
                if jj >= AV_DELAY:
                    flush_av(av1, den1, 1, jcs1, jj - AV_DELAY + 1)
            flush_av(av1, den1, 1, jcs1, len(jcs1))
            out_proj_half(1, 3)

            # ---- tail ----
            in_tail[0] = True
            for b in range(B):
                normalize(av1, den1, 1, b)
            for b in range(B):
                for ib in range(4, 8):
                    out_proj_half(b, ib)

    nc.compile()
    _NC_CACHE = nc
    return nc


def _prep_inputs(x, mem_k, mem_v, pos_bias, Wq, Wkv, Wo):
    """Build per-core input maps (host-side sharding)."""
    x = np.ascontiguousarray(x, dtype=np.float32)
    xT = np.ascontiguousarray(x.transpose(0, 2, 1)).astype(np.float16)

    import ml_dtypes
    pb = np.ascontiguousarray(
        pos_bias[0].transpose(0, 2, 1)).astype(np.float32)     # [16, JT, N]
    jj = np.arange(JT)[:, None]
    ii = np.arange(N)[None, :]
    mask = jj > (ii + M)
    eb0 = np.where(mask[None], np.float32(-240.0), pb).astype(
        ml_dtypes.float8_e4m3fn)
    ebx1 = np.where(mask[None], np.float32(0.0), np.exp(pb)).astype(
        ml_dtypes.bfloat16)

    def shuffle_w(w):  # [1024, 128] -> [128, 1024] with kc-major columns
        return np.ascontiguousarray(
            w.reshape(NKC, 128, 128).transpose(1, 0, 2).reshape(128, DIM))

    in_maps = []
    for c in range(NCORE):
        cs = 128 * c
        wq = shuffle_w(np.asarray(Wq[:, cs:cs + 128] * SCALE)).astype(np.float16)
        wk = shuffle_w(np.asarray(Wkv[:, cs:cs + 128])).astype(np.float16)
        wv = shuffle_w(
            np.asarray(Wkv[:, DIM + cs:DIM + cs + 128])).astype(np.float16)
        wo = np.ascontiguousarray(Wo[cs:cs + 128, :]).astype(np.float16)
        mkT = np.ascontiguousarray(
            mem_k[:, :, cs:cs + 128].transpose(0, 2, 1)).astype(np.float16)
        mv_s = mem_v[:, :, cs:cs + 128].astype(np.float16).reshape(
            B, NJ_MEM, 128, HPC, DH)
        mv = np.empty((B, HPC, 128, NJ_MEM, DH), dtype=np.float16)
        for h in range(HPC):
            mv[:, h] = mv_s[:, :, :, h, :].transpose(0, 2, 1, 3)
        mv = mv.reshape(B, HPC, 128, NJ_MEM * DH)
        in_maps.append({
            "xT": xT,
            "wq": wq, "wk": wk, "wv": wv, "wo": wo,
            "mkT": mkT,
            "mv": np.ascontiguousarray(mv),
            "eb0": np.ascontiguousarray(eb0[2 * c]),
            "ebx1": np.ascontiguousarray(ebx1[2 * c + 1]),
        })
    return in_maps


def kernel(x, mem_k, mem_v, pos_bias, Wq, Wkv, Wo, bo, **_kw):
    nc = _build()
    in_maps = _prep_inputs(
        np.asarray(x), np.asarray(mem_k), np.asarray(mem_v),
        np.asarray(pos_bias), np.asarray(Wq), np.asarray(Wkv), np.asarray(Wo))
    res = bass_utils.run_bass_kernel_spmd(nc, in_maps, core_ids=list(range(NCORE)))
    out = np.zeros((B, N, DIM), dtype=np.float64)
    for r in res.results:
        out += r["out"].astype(np.float64)
    out += np.asarray(bo, dtype=np.float64)[None, None, :]
    return out.astype(np.float32)


# revision 33
# speedup vs baseline: 1.0553x; 1.0158x over previous
"""Trainium2 Bass kernel for memory-augmented causal attention.

Reference computation (b=2, n=1024, m=1024 memory, 16 heads, d_head=64):
  q = (x @ Wq) * scale ; k,v = split(x @ Wkv) ; k = [mem_k; k] ; v = [mem_v; v]
  sim = q k^T + pos_bias ; causal mask on self part ; softmax ; out = attn v
  return out @ Wo + bo

Sharding: 16 heads across 8 cores (2 heads/core), both batches on every core.
Each core computes a partial output (its heads' contribution through Wo rows);
host sums the 8 partials.

Structure (wave-packed):
  - The PE array is treated as a 4x4 grid of 32x32 subarrays; matmuls whose
    (contraction-rows x output-partition) rectangles are disjoint execute
    concurrently (tile_position row/col tiling).  Per j-chunk the attention
    work is emitted as waves of concurrent MMs:
      qk wave (per b):  h0 [rows 0-63 x cols all] || h1 [rows 64-127 x ...]
        (memory chunks: h1 split jlo/jhi with the output partition halves
        swapped -- the "rotation" -- so the bias wave can 4-pack)
      bias wave (per b, memory chunks): 4 concurrent 64-row identity MMs in
        the 4 disjoint diagonal/anti-diagonal rectangles; self chunks fall
        back to 2 serial full-identity MMs (no rotation there, so the
        on-chip self-v transpose copy stays partition-aligned).
      AV wave (per b):  h0 -> av[0:64] (col strips 0-1) || h1 -> av[64:128]
      denom wave: 4 concurrent [128,1]-stationary ones-MMs at output rows
        0/32/64/96 of one PSUM bank (softmax denominators).
  - pos_bias is fp8e4 (mask=-240): half DMA, exact-enough bias add on PE.
  - Causal i-windowing: self-attention chunks skip their masked i-prefix.
  - Normalization: reciprocal of denom rows, gpsimd partition-broadcast,
    one [128,512] DVE multiply writing outT directly (no staging DMA).
  - Output projection in f16 (outT, Wo); partial output summed on host.
  - PSUM: sim ring 2x[128,1024] (4 banks) + av 2x[128,512] (2) + denom
    2x[128,512] (2); transients share the sim ring.
"""

import numpy as np

import concourse.bass as bass
import concourse.mybir as mybir
import concourse.tile as tile
from concourse import bacc
from concourse import bass_utils
from concourse.masks import make_identity

F32 = mybir.dt.float32
F32R = mybir.dt.float32r
F16 = mybir.dt.float16
BF16 = mybir.dt.bfloat16
F8E4 = mybir.dt.float8e4

HEADS = 16
DH = 64               # head dim
B = 2                 # batch
N = 1024              # query length
M = 1024              # memory length
JT = N + M            # total key length
DIM = 1024
SCALE = DH ** -0.5
NCORE = 8
HPC = HEADS // NCORE  # heads per core = 2

NKC = DIM // 128      # contraction chunks for projections = 8
NJ = JT // 128        # j chunks = 16
NJ_MEM = M // 128     # memory j chunks = 8
NIC = N // 512        # i chunks of 512 = 2

AV_DELAY = 3          # AV trails sims by this many j-chunks


def _self_chunks(ic):
    return (ic * 512 + 511) // 128 + 1


def _unmasked_jcs(ic):
    return list(range(NJ_MEM)) + [NJ_MEM + k for k in range(min(8, _self_chunks(ic)))]


WINDOWING = True


def _wlo(ic, jc):
    """first unmasked i-column (within the 512 i-chunk) for this j-chunk"""
    if not WINDOWING or jc < NJ_MEM:
        return 0
    return max(0, (jc - NJ_MEM) * 128 - ic * 512)


_NC_CACHE = None


def _build():
    global _NC_CACHE
    if _NC_CACHE is not None:
        return _NC_CACHE

    nc = bacc.Bacc("TRN2", target_bir_lowering=False, debug=False)

    XT = nc.dram_tensor("xT", [B, DIM, N], BF16, kind="ExternalInput").ap()
    WQ = nc.dram_tensor("wq", [128, DIM], F16, kind="ExternalInput").ap()
    WK = nc.dram_tensor("wk", [128, DIM], F16, kind="ExternalInput").ap()
    WV = nc.dram_tensor("wv", [128, DIM], F16, kind="ExternalInput").ap()
    WO = nc.dram_tensor("wo", [128, DIM], F16, kind="ExternalInput").ap()
    MKT = nc.dram_tensor("mkT", [B, 128, M], F16, kind="ExternalInput").ap()
    MV = nc.dram_tensor("mv", [B, HPC, 128, NJ_MEM * DH], F16,
                        kind="ExternalInput").ap()
    EB0 = nc.dram_tensor("eb0", [JT, N], F8E4, kind="ExternalInput").ap()
    EBX1 = nc.dram_tensor("ebx1", [JT, N], BF16, kind="ExternalInput").ap()
    OUT = nc.dram_tensor("out", [B, N, DIM], BF16, kind="ExternalOutput").ap()

    with tile.TileContext(nc) as tc:
        with tc.tile_pool(name="const", bufs=1) as cp, \
             tc.tile_pool(name="wts", bufs=1) as wp, \
             tc.tile_pool(name="xtp", bufs=16) as xtp, \
             tc.tile_pool(name="big", bufs=1) as bigp, \
             tc.tile_pool(name="stage", bufs=2) as stp, \
             tc.tile_pool(name="ebp", bufs=8) as ebp, \
             tc.tile_pool(name="expp", bufs=10) as expp, \
             tc.tile_pool(name="outst", bufs=4) as outstp, \
             tc.tile_pool(name="smalls", bufs=2) as smallp, \
             tc.tile_pool(name="psum", bufs=1, space="PSUM") as psp:

            # ---- input DMAs, priority order, all on the sync HW-DGE ----
            wq_t = wp.tile([128, DIM], F16)
            wk_t = wp.tile([128, DIM], F16)
            wv_t = wp.tile([128, DIM], F16)
            wo_t = wp.tile([128, DIM], F16)
            # first kc chunk of wq lands first: matmul #1 needs only it
            nc.sync.dma_start(wq_t[:, 0:128], WQ[:, 0:128])
            nc.sync.dma_start(wq_t[:, 128:DIM], WQ[:, 128:DIM])
            nc.sync.dma_start(wk_t[:, 0:128], WK[:, 0:128])
            xbig = {}
            xts = {}
            for b in range(B):
                for g in range(NKC // 2):
                    xbig[(b, g)] = xtp.tile([128, 2 * N], BF16,
                                            name=f"xb{b}_{g}", tag="xt")
                for kc in range(NKC):
                    xts[(b, kc)] = xbig[(b, kc // 2)][
                        :, (kc % 2) * N:(kc % 2 + 1) * N]

            def load_x(b, g):
                nc.sync.dma_start(
                    xbig[(b, g)][:].rearrange("p (c i) -> p c i", c=2),
                    XT[b, g * 256:(g + 1) * 256, :].rearrange(
                        "(c p) i -> p c i", c=2))

            for g in range(NKC // 2):
                load_x(0, g)
            nc.sync.dma_start(wk_t[:, 128:DIM], WK[:, 128:DIM])
            for g in range(NKC // 2):
                load_x(1, g)
            nc.sync.dma_start(wv_t[:], WV)

            qT = [bigp.tile([128, N], F16, name=f"qT{b}") for b in range(B)]
            kT = [bigp.tile([128, JT], F16, name=f"kT{b}") for b in range(B)]
            vaug = [bigp.tile([128, HPC * NJ * DH], F16, name=f"vaug{b}")
                    for b in range(B)]
            outT = [bigp.tile([128, N], F16, name=f"outT{b}") for b in range(B)]

            def vaug64(b, h, jc):
                o = (h * NJ + jc) * DH
                return vaug[b][:, o:o + DH]

            for b in range(B):
                nc.sync.dma_start(kT[b][:, 0:M], MKT[b])
            ebt = {}
            ebbig = {}
            for g in range(4):
                ebbig[(0, g)] = ebp.tile([128, 4 * N], F8E4,
                                         name=f"ebb0_{g}", tag="eb0",
                                         bufs=4)
                ebbig[(1, g)] = ebp.tile([128, 4 * N], BF16,
                                         name=f"ebb1_{g}", tag="eb1",
                                         bufs=4)
            for h in range(HPC):
                for jc in range(NJ):
                    ebt[(h, jc)] = ebbig[(h, jc // 4)][
                        :, (jc % 4) * N:(jc % 4 + 1) * N]

            def load_eb(g):
                nc.sync.dma_start(
                    ebbig[(0, g)][:].rearrange("p (c i) -> p c i", c=4),
                    EB0[g * 512:(g + 1) * 512, :].rearrange(
                        "(c p) i -> p c i", c=4))
                nc.sync.dma_start(
                    ebbig[(1, g)][:].rearrange("p (c i) -> p c i", c=4),
                    EBX1[g * 512:(g + 1) * 512, :].rearrange(
                        "(c p) i -> p c i", c=4))

            load_eb(0)
            load_eb(1)
            for b in range(B):
                for h in range(HPC):
                    nc.sync.dma_start(
                        vaug[b][:, h * NJ * DH:(h * NJ + NJ_MEM) * DH],
                        MV[b, h])
            load_eb(2)
            load_eb(3)
            nc.sync.dma_start(wo_t[:], WO)

            # ---- constants (before memsets: warms gate on identh) ----
            identh = cp.tile([128, 128], F16)
            make_identity(nc, identh[:])
            identb = cp.tile([128, 128], F8E4)
            make_identity(nc, identb[:])
            # ones column for the denominator matmuls
            onesc = cp.tile([128, 1], F16)
            nc.vector.memset(onesc[:], 1.0)
            # all-ones [128, 64] for the reciprocal broadcast outer-products
            onesb = cp.tile([128, 64], F32)
            nc.vector.memset(onesb[:], 1.0)

            def warm(n):
                wps = psp.tile([128, 512], F32, name="warmps", tag="simps",
                               bufs=2)
                for _ in range(n):
                    nc.tensor.matmul(wps[:, 0:128], identh[:], identh[:],
                                     start=True, stop=True,
                                     skip_group_check=True)

            # =============== Phase A: q/k projections ===============
            def proj_qk(kind, b):
                wt = wq_t if kind == "q" else wk_t
                ps = psp.tile([128, N], F32, name="projps", tag="simps", bufs=2)
                # kc-outer: consume each xT tile as it lands (both i-halves
                # per tile) so the PE trails the DMA stream instead of
                # catching up and stalling
                for kc in range(NKC):
                    for icx in range(NIC):
                        nc.tensor.matmul(
                            ps[:, icx * 512:(icx + 1) * 512],
                            wt[:, kc * 128:(kc + 1) * 128],
                            xts[(b, kc)][:, icx * 512:(icx + 1) * 512],
                            start=(kc == 0), stop=(kc == NKC - 1))
                if kind == "q":
                    nc.vector.tensor_copy(qT[b][:], ps[:])
                else:
                    nc.vector.tensor_copy(kT[b][:, M:JT], ps[:])

            warm(24)
            for kind, b in (("q", 0), ("k", 0), ("q", 1), ("k", 1)):
                proj_qk(kind, b)

            # ---- v projection pieces (hooked under early ic0 chunks) ----
            vst = [stp.tile([128, N], F16, name=f"vstage{b}") for b in range(B)]

            def v_proj_piece(b, icx):
                ps = psp.tile([128, 512], F32, name="vps", tag="simps",
                              bufs=2)
                for kc in range(NKC):
                    nc.tensor.matmul(
                        ps[:],
                        wv_t[:, kc * 128:(kc + 1) * 128],
                        xts[(b, kc)][:, icx * 512:(icx + 1) * 512],
                        start=(kc == 0), stop=(kc == NKC - 1))
                copy_balanced(vst[b][:, icx * 512:(icx + 1) * 512], ps[:],
                              eng="v")

            def v_transpose(b):
                for jb in range(8):
                    tp = psp.tile([128, 128], F16, name="tps",
                                  tag="simps", bufs=2)
                    nc.tensor.transpose(
                        tp[:], vst[b][:, jb * 128:(jb + 1) * 128], identh[:])
                    jc = NJ_MEM + jb
                    dst = vaug[b][:].rearrange(
                        "p (hh jjc x) -> p hh jjc x", hh=HPC, x=DH)[
                        :, :, jc, :]
                    nc.vector.tensor_copy(
                        dst, tp[:].rearrange("p (hh x) -> p hh x", hh=HPC))

            copy_idx = 0
            in_tail = [False]

            def copy_balanced(out_ap, in_ap, eng=None):
                nonlocal copy_idx
                if eng is None:
                    eng = "v" if copy_idx % 2 == 0 else "s"
                    copy_idx += 1
                if eng == "v":
                    nc.vector.tensor_copy(out_ap, in_ap)
                else:
                    nc.scalar.copy(out_ap, in_ap)

            # =============== attention machinery ===============
            def normalize(av, den, ic, b):
                # denominator rows: (h, b) -> b*64 + h*32
                for h in range(HPC):
                    r = b * 64 + h * 32
                    s = smallp.tile([1, 512], F32, name=f"sums{h}")
                    nc.vector.tensor_copy(s[:], den[r:r + 1, :])
                    rc = smallp.tile([1, 512], F32, name=f"recip{h}")
                    nc.vector.reciprocal_approx_fast(rc[:], s[:])
                    rb = smallp.tile([64, 512], F32, name=f"recipb{h}")
                    nc.gpsimd.partition_broadcast(rb[:], rc[:])
                    nc.vector.tensor_tensor(
                        outT[b][h * 64:(h + 1) * 64,
                                ic * 512:(ic + 1) * 512],
                        av[b][h * 64:(h + 1) * 64, :], rb[:],
                        mybir.AluOpType.mult)

            def out_proj_half(b, ib):
                ob = outstp.tile([128, DIM], BF16, name="ob")
                ps = psp.tile([128, N], F32, name="ops", tag="simps",
                              bufs=2)
                for dc in range(DIM // 512):
                    nc.tensor.matmul(
                        ps[:, dc * 512:(dc + 1) * 512],
                        outT[b][:, ib * 128:(ib + 1) * 128],
                        wo_t[:, dc * 512:(dc + 1) * 512],
                        start=True, stop=True)
                copy_balanced(ob[:], ps[:])
                nc.sync.dma_start(OUT[b, ib * 128:(ib + 1) * 128, :], ob[:])

            def emit_sims(ic, jc, jj):
                wlo = _wlo(ic, jc)
                sb = [psp.tile([128, N], F32, name=f"simps{b}", tag="simps",
                               bufs=2) for b in range(B)]
                jcl = jc * 128
                for b in range(B):
                    qsl = slice(ic * 512 + wlo, (ic + 1) * 512)
                    # wave: disjoint row strips, same tile -> co-ready pair
                    nc.tensor.matmul(sb[b][:, wlo:512],
                                     kT[b][0:64, jcl:jcl + 128],
                                     qT[b][0:64, qsl],
                                     start=True, stop=False,
                                     skip_group_check=True)
                    nc.tensor.matmul(sb[b][:, 512 + wlo:1024],
                                     kT[b][64:128, jcl:jcl + 128],
                                     qT[b][64:128, qsl],
                                     start=True, stop=True,
                                     skip_group_check=True)
                    ec = slice(ic * 512 + wlo, (ic + 1) * 512)
                    nc.tensor.matmul(sb[b][:, wlo:512], identb[:],
                                     ebt[(0, jc)][:, ec],
                                     start=False, stop=True,
                                     skip_group_check=True)
                out = []
                ec = slice(ic * 512 + wlo, (ic + 1) * 512)
                for b in range(B):
                    er = expp.tile([128, N], F16, name="expraw", tag="expraw")
                    if wlo == 0:
                        nc.scalar.activation(
                            er[:], sb[b][:], mybir.ActivationFunctionType.Exp)
                    else:
                        nc.scalar.activation(
                            er[:].rearrange("p (h i) -> p h i",
                                            h=HPC)[:, :, wlo:512],
                            sb[b][:].rearrange("p (h i) -> p h i",
                                               h=HPC)[:, :, wlo:512],
                            mybir.ActivationFunctionType.Exp)
                    # h1 bias applied multiplicatively (exp(sim)*exp(bias))
                    nc.vector.tensor_tensor(
                        er[:, 512 + wlo:1024], er[:, 512 + wlo:1024],
                        ebt[(1, jc)][:, ec], mybir.AluOpType.mult)
                    out.append(er)
                return out

            def make_av():
                avs = [psp.tile([128, 512], F32, name=f"avc{b}", tag="avps",
                                bufs=2) for b in range(B)]
                for a in avs:
                    nc.vector.memset(a[:], 0.0)
                return avs

            def make_denom():
                d = psp.tile([128, 512], F32, name="denom", tag="denom",
                             bufs=2)
                nc.vector.memset(d[:], 0.0)
                return d

            def emit_av(av, den, jc, ic, e2s, first, last):
                wlo = _wlo(ic, jc)
                for b in range(B):
                    # wave: disjoint col strips -> concurrent
                    nc.tensor.matmul(av[b][0:64, wlo:512], vaug64(b, 0, jc),
                                     e2s[b][:, wlo:512],
                                     start=False, stop=last,
                                     skip_group_check=True)
                    nc.tensor.matmul(av[b][64:128, wlo:512], vaug64(b, 1, jc),
                                     e2s[b][:, 512 + wlo:1024],
                                     start=False, stop=last,
                                     skip_group_check=True)
                # denominator wave: 4 concurrent single-row MMs
                for b in range(B):
                    for h in range(HPC):
                        r = b * 64 + h * 32
                        nc.tensor.matmul(
                            den[r:r + 1, wlo:512], onesc[:, 0:1],
                            e2s[b][:, h * 512 + wlo:(h + 1) * 512],
                            start=False, stop=last,
                            skip_group_check=True, tile_position=(0, r))

            pend = []
            done_av = 0

            def flush_av(av, den, ic, jcs, upto):
                nonlocal done_av
                while done_av < upto:
                    jc, e2s = pend[done_av]
                    emit_av(av, den, jc, ic, e2s, done_av == 0,
                            done_av == len(jcs) - 1)
                    done_av += 1

            # ---- ic = 0 (first chunks fill the exp pipeline while
            #      the v projection runs on the PE) ----
            jcs0 = _unmasked_jcs(0)
            pend = []
            done_av = 0
            av0 = make_av()
            den0 = make_denom()
            for jj, jc in enumerate(jcs0):
                pend.append((jc, emit_sims(0, jc, jj)))
                if jj == 1:
                    for b in range(B):
                        for icx in range(NIC):
                            v_proj_piece(b, icx)
                        v_transpose(b)
                if jj >= AV_DELAY:
                    flush_av(av0, den0, 0, jcs0, jj - AV_DELAY + 1)
            flush_av(av0, den0, 0, jcs0, len(jcs0))

            # ---- ic = 1 (ic0 normalize + out-proj interleaved) ----
            jcs1 = _unmasked_jcs(1)
            pend = []
            done_av = 0
            av1 = None
            den1 = None
            normalize(av0, den0, 0, 0)
            for jj, jc in enumerate(jcs1):
                pend.append((jc, emit_sims(1, jc, jj)))
                if jj == 0:
                    av1 = make_av()
                    den1 = make_denom()
                    normalize(av0, den0, 0, 1)
                if jj >= 3 and jj % 2 == 1:
                    i = (jj - 3) // 2
                    out_proj_half(i // 4, i % 4)
                if jj >= AV_DELAY:
                    flush_av(av1, den1, 1, jcs1, jj - AV_DELAY + 1)
            flush_av(av1, den1, 1, jcs1, len(jcs1))
            out_proj_half(1, 3)

            # ---- tail ----
            in_tail[0] = True
            for b in range(B):
                normalize(av1, den1, 1, b)
            for b in range(B):
                for ib in range(4, 8):
                    out_proj_half(b, ib)

    nc.compile()
    _NC_CACHE = nc
    return nc


def _prep_inputs(x, mem_k, mem_v, pos_bias, Wq, Wkv, Wo):
    """Build per-core input maps (host-side sharding)."""
    import ml_dtypes as _mld
    x = np.ascontiguousarray(x, dtype=np.float32)
    xT = np.ascontiguousarray(x.transpose(0, 2, 1)).astype(_mld.bfloat16)

    import ml_dtypes
    pb = np.ascontiguousarray(
        pos_bias[0].transpose(0, 2, 1)).astype(np.float32)     # [16, JT, N]
    jj = np.arange(JT)[:, None]
    ii = np.arange(N)[None, :]
    mask = jj > (ii + M)
    eb0 = np.where(mask[None], np.float32(-240.0), pb).astype(
        ml_dtypes.float8_e4m3fn)
    ebx1 = np.where(mask[None], np.float32(0.0), np.exp(pb)).astype(
        ml_dtypes.bfloat16)

    def shuffle_w(w):  # [1024, 128] -> [128, 1024] with kc-major columns
        return np.ascontiguousarray(
            w.reshape(NKC, 128, 128).transpose(1, 0, 2).reshape(128, DIM))

    in_maps = []
    for c in range(NCORE):
        cs = 128 * c
        wq = shuffle_w(np.asarray(Wq[:, cs:cs + 128] * SCALE)).astype(np.float16)
        wk = shuffle_w(np.asarray(Wkv[:, cs:cs + 128])).astype(np.float16)
        wv = shuffle_w(
            np.asarray(Wkv[:, DIM + cs:DIM + cs + 128])).astype(np.float16)
        wo = np.ascontiguousarray(Wo[cs:cs + 128, :]).astype(np.float16)
        mkT = np.ascontiguousarray(
            mem_k[:, :, cs:cs + 128].transpose(0, 2, 1)).astype(np.float16)
        mv_s = mem_v[:, :, cs:cs + 128].astype(np.float16).reshape(
            B, NJ_MEM, 128, HPC, DH)
        mv = np.empty((B, HPC, 128, NJ_MEM, DH), dtype=np.float16)
        for h in range(HPC):
            mv[:, h] = mv_s[:, :, :, h, :].transpose(0, 2, 1, 3)
        mv = mv.reshape(B, HPC, 128, NJ_MEM * DH)
        in_maps.append({
            "xT": xT,
            "wq": wq, "wk": wk, "wv": wv, "wo": wo,
            "mkT": mkT,
            "mv": np.ascontiguousarray(mv),
            "eb0": np.ascontiguousarray(eb0[2 * c]),
            "ebx1": np.ascontiguousarray(ebx1[2 * c + 1]),
        })
    return in_maps


def kernel(x, mem_k, mem_v, pos_bias, Wq, Wkv, Wo, bo, **_kw):
    nc = _build()
    in_maps = _prep_inputs(
        np.asarray(x), np.asarray(mem_k), np.asarray(mem_v),
        np.asarray(pos_bias), np.asarray(Wq), np.asarray(Wkv), np.asarray(Wo))
    res = bass_utils.run_bass_kernel_spmd(nc, in_maps, core_ids=list(range(NCORE)))
    out = np.zeros((B, N, DIM), dtype=np.float64)
    for r in res.results:
        out += r["out"].astype(np.float64)
    out += np.asarray(bo, dtype=np.float64)[None, None, :]
    return out.astype(np.float32)
